# revision 16
# baseline (speedup 1.0000x reference)
"""GQA attention + RoPE + O-proj, tensor-parallel over 8 NeuronCores.

Strategy (head-parallel TP + all-to-all reshard before O-proj):
  - host: transpose x -> xT [DIM, T] in bf16; shuffle per-head wq/wk columns
    to [even hd | odd hd] so RoPE works in the transposed layout; weights in
    bf16 (matmul rate is unchanged vs fp32r, DMA halves).
  - core c: projects q for heads {2c, 2c+1} and k,v for kv-head c//2 over
    all tokens (bf16 weight-stationary matmuls, xT streamed in quarter
    tiles with 2KB lines), applies RoPE inline per token-pair in fp32r,
    transposes V inline, then runs causal attention in S^T [k, q] layout
    with no-max softmax; denominator strip-adds run on the Pool engine
    (DVE was the attention-phase bottleneck), cross-partition sums via
    ones-matmul + fast Newton reciprocal.
  - Two AllToAlls (one per local head, bf16 payload, overlapped with
    attention) reshard attention outputs head-major -> token-sharded; each
    core then computes its 512 output rows against the full wo (bf16,
    fully prefetched during projection/attention).
"""

import os
import numpy as np
import ml_dtypes

import concourse.bass as bass
import concourse.bacc as bacc
import concourse.tile as tile
from concourse import mybir
from concourse.bass_utils import run_bass_kernel_spmd

F32 = mybir.dt.float32
F32R = mybir.dt.float32r
BF16 = mybir.dt.bfloat16
NPBF16 = ml_dtypes.bfloat16

N_CORES = 8

# Full-problem config (hardcoded per spec).
B, SB, DIM = 2, 2048, 2048         # batches, seq per batch, model dim
H, HKV, HD = 16, 4, 128            # q heads, kv heads, head dim
SCALE = 1.0 / float(np.sqrt(HD))

T = B * SB                          # 4096 flat tokens (batch-major)
TPC = T // N_CORES                  # 512 tokens per core (output shard)
HPC = H // N_CORES                  # 2 q heads per core
QW = HPC * HD                       # 256 q cols per core
NKD = DIM // 128                    # 16 contraction tiles for projections
NG = SB // 512                      # 4 q-groups of 512 per batch
KT = SB // 128                      # 16 k-tiles per batch
NTT = T // 128                      # 32 token tiles total
NHD = (H * HD) // 128               # 16 hd row-tiles of wo


def _build():
    nc = bacc.Bacc("TRN2", target_bir_lowering=False, debug=False,
                   num_devices=N_CORES)

    xT = nc.dram_tensor("xT", [DIM, T], BF16, kind="ExternalInput").ap()
    wq_c = nc.dram_tensor("wq_c", [DIM, QW], BF16, kind="ExternalInput").ap()
    wk_c = nc.dram_tensor("wk_c", [DIM, HD], BF16, kind="ExternalInput").ap()
    wv_c = nc.dram_tensor("wv_c", [DIM, HD], BF16, kind="ExternalInput").ap()
    wo_f = nc.dram_tensor("wo_f", [H * HD, DIM], BF16, kind="ExternalInput").ap()
    cosd = nc.dram_tensor("cosd", [128, SB], F32, kind="ExternalInput").ap()
    sind = nc.dram_tensor("sind", [128, SB], F32, kind="ExternalInput").ap()
    sgn = nc.dram_tensor("sgn", [128, 1], F32, kind="ExternalInput").ap()
    tri = nc.dram_tensor("tri", [128, 512], F32, kind="ExternalInput").ap()
    ones = nc.dram_tensor("ones", [128, 128], F32R, kind="ExternalInput").ap()
    ident = nc.dram_tensor("ident", [128, 128], BF16, kind="ExternalInput").ap()
    out_c = nc.dram_tensor("out_c", [TPC, DIM], F32, kind="ExternalOutput").ap()

    a2a_in = []
    a2a_out = []
    for hl in range(HPC):
        a2a_in.append(nc.dram_tensor(f"a2a_in{hl}",
                                     [N_CORES, HD, TPC], BF16).ap())
        a2a_out.append(nc.dram_tensor(f"a2a_out{hl}",
                                      [N_CORES, HD, TPC], BF16).ap())

    SEG = min(512, SB)             # rope segment (never crosses a batch)
    NKQ = max(1, NKD // 4)         # dim-tiles per xt quarter
    NQT = NKD // NKQ               # quarters per token group-pair

    DQH = DIM // 2
    with tile.TileContext(nc) as tc:
        wop = tc.alloc_tile_pool(name="wop", bufs=1)
        wo3 = wo_f.rearrange("(n p) m -> p n m", p=128)      # [128,NHD,DIM]
        wo_h0 = wop.tile([128, NHD * DQH], BF16, tag="wo0")
        wo_h03 = wo_h0.rearrange("p (n m) -> p n m", n=NHD)

        def wo_slice(kk, c0, c1):
            # columns [c0, c1) of wo hd-tile kk, across the two halves
            if c1 <= DQH:
                return wo_h03[:, kk, c0:c1]
            return wo_h13[:, kk, c0 - DQH:c1 - DQH]

        with tc.tile_pool(name="const", bufs=1) as constp, \
             tc.tile_pool(name="qkv", bufs=1) as qkvp:
            sgn_sb = constp.tile([128, 1], F32)
            nc.sync.dma_start(sgn_sb[:], sgn[:, :])

            # persistent roped projections + V in natural layout
            qT0 = qkvp.tile([128, T], F32R, tag="qT0")
            qT1 = qkvp.tile([128, T], F32R, tag="qT1")
            kT = qkvp.tile([128, T], F32R, tag="kT")
            vT = qkvp.tile([128, T], BF16, tag="vT")
            chunks = [qT0, qT1, kT]

            # ------ phase 1: projections + inline RoPE + V transpose ------
            with tc.tile_pool(name="w", bufs=1) as wp, \
                 tc.tile_pool(name="cs", bufs=1) as csp, \
                 tc.tile_pool(name="xt", bufs=4) as xtp, \
                 tc.tile_pool(name="rtmp", bufs=1) as rp, \
                 tc.tile_pool(name="pproj", bufs=1, space="PSUM") as pp:
                wq_sb = wp.tile([128, NKD * QW], BF16)
                wk_sb = wp.tile([128, NKD * HD], BF16)
                wv_sb = wp.tile([128, NKD * HD], BF16)
                # chunked weight loads: kk group 0 lands first so the
                # first matmuls don't wait on whole-tensor DMAs
                wq3d = wq_sb.rearrange("p (n m) -> p n m", n=NKD)
                wk3d = wk_sb.rearrange("p (n m) -> p n m", n=NKD)
                wv3d = wv_sb.rearrange("p (n m) -> p n m", n=NKD)
                wqs = wq_c.rearrange("(n p) m -> p n m", p=128)
                wks = wk_c.rearrange("(n p) m -> p n m", p=128)
                wvs = wv_c.rearrange("(n p) m -> p n m", p=128)
                NWCH = 4
                WCH = NKD // NWCH
                for ch in range(NWCH):
                    k0, k1 = ch * WCH, (ch + 1) * WCH
                    nc.sync.dma_start(wq3d[:, k0:k1, :], wqs[:, k0:k1, :])
                    nc.sync.dma_start(wk3d[:, k0:k1, :], wks[:, k0:k1, :])
                    nc.sync.dma_start(wv3d[:, k0:k1, :], wvs[:, k0:k1, :])

                def w_slice(c, kk):
                    if c < 2:
                        return wq_sb[:, kk * QW + c * 128: kk * QW + (c + 1) * 128]
                    if c == 2:
                        return wk_sb[:, kk * HD:(kk + 1) * HD]
                    return wv_sb[:, kk * HD:(kk + 1) * HD]

                xT3 = xT.rearrange("(n p) m -> p n m", p=128)  # [128,NKD,T]
                npair = T // 1024
                cos_sb = sin_sb = None
                for p in range(npair):
                    # stream this pair's xT as quarter tiles, 1024 tokens
                    # wide (2KB bf16 lines, kk-major use)
                    xts = []            # [quarter] -> [128, NKQ*1024]
                    for q in range(NQT):
                        xt_q = xtp.tile([128, NKQ * 1024], BF16, tag="xt")
                        nc.sync.dma_start(
                            xt_q.rearrange("p (n m) -> p n m", n=NKQ),
                            xT3[:, q * NKQ:(q + 1) * NKQ,
                                p * 1024:(p + 1) * 1024])
                        xts.append(xt_q)
                    if cos_sb is None:
                        cos_sb = csp.tile([128, SB], F32)
                        nc.sync.dma_start(cos_sb[:], cosd[:, :])
                        sin_sb = csp.tile([128, SB], F32)
                        nc.sync.dma_start(sin_sb[:], sind[:, :])
                    pss = []
                    for c in range(4):
                        ps_c = pp.tile([128, 1024], F32, tag=f"pp{c}")
                        pss.append(ps_c)
                    for kk in range(NKD):
                        for c in range(4):
                            lhsT = w_slice(c, kk)
                            xt_q = xts[kk // NKQ]
                            base = (kk % NKQ) * 1024
                            for j in (0, 1):
                                nc.tensor.matmul(
                                    pss[c][:, j * 512:(j + 1) * 512], lhsT,
                                    xt_q[:, base + j * 512: base + (j + 1) * 512],
                                    start=(kk == 0), stop=(kk == NKD - 1))
                    # drain q0/q1/k with RoPE staged below; v via transpose
                    cp0 = 1024 * p
                    for c in range(3):
                        nc.vector.tensor_copy(
                            chunks[c][:, cp0:cp0 + 1024], pss[c][:])
                    nc.vector.tensor_copy(vT[:, cp0:cp0 + 1024], pss[3][:])
                    # prefetch part of wo's first column-half per pair
                    w0 = p * (NHD // npair)
                    w1 = (p + 1) * (NHD // npair)
                    nc.sync.dma_start(wo_h03[:, w0:w1, :],
                                      wo3[:, w0:w1, 0:DQH])
                    # RoPE on the pair's columns, per batch segment
                    for s0 in range(cp0, cp0 + 1024, SEG):
                        pos0 = s0 % SB
                        for X in chunks:
                            tcs = rp.tile([128, SEG], F32, tag="tc")
                            nc.vector.tensor_tensor(
                                tcs[:], X[:, s0:s0 + SEG],
                                cos_sb[:, pos0:pos0 + SEG],
                                op=mybir.AluOpType.mult)
                            tsn = rp.tile([128, SEG], F32, tag="ts")
                            nc.vector.tensor_tensor(
                                tsn[:], X[:, s0:s0 + SEG],
                                sin_sb[:, pos0:pos0 + SEG],
                                op=mybir.AluOpType.mult)
                            tsw = rp.tile([128, SEG], F32, tag="tw")
                            nc.sync.dma_start(tsw[0:64, :], tsn[64:128, :])
                            nc.sync.dma_start(tsw[64:128, :], tsn[0:64, :])
                            # X = tcs + sgn * tsw   (sgn = -1 top / +1 bottom)
                            nc.vector.scalar_tensor_tensor(
                                X[:, s0:s0 + SEG], tsw[:], sgn_sb[:, 0:1],
                                tcs[:], op0=mybir.AluOpType.mult,
                                op1=mybir.AluOpType.add)

            # ---------------- phase 3: attention ----------------------
            DQ = DIM // 4
            with tc.tile_pool(name="att", bufs=2) as ap, \
                 tc.tile_pool(name="attc", bufs=1) as apc, \
                 tc.tile_pool(name="pstr", bufs=2) as pstr, \
                 tc.tile_pool(name="psS", bufs=2, space="PSUM") as psS, \
                 tc.tile_pool(name="psO", bufs=1, space="PSUM") as psO:
                tri_sb = apc.tile([128, 512], F32)
                nc.sync.dma_start(tri_sb[:], tri[:, :])
                ones_sb = apc.tile([128, 128], F32R)
                nc.sync.dma_start(ones_sb[:], ones[:, :])
                ident_sb = apc.tile([128, 128], BF16)
                nc.sync.dma_start(ident_sb[:], ident[:, :])
                # second wo column-half (xt stream pool has freed by now)
                wo_h1 = wop.tile([128, NHD * DQH], BF16, tag="wo1")
                wo_h13 = wo_h1.rearrange("p (n m) -> p n m", n=NHD)
                for ch in range(4):
                    w0 = ch * (NHD // 4)
                    w1 = (ch + 1) * (NHD // 4)
                    nc.sync.dma_start(wo_h13[:, w0:w1, :],
                                      wo3[:, w0:w1, DQH:DIM])
                Vt = qkvp.tile([128, T], F32R, tag="Vt")
                for ttg in range(NTT):
                    psv = psS.tile([128, 128], BF16, tag="S")
                    nc.tensor.transpose(psv[:],
                                        vT[:, ttg * 128:(ttg + 1) * 128],
                                        ident_sb[:])
                    nc.vector.tensor_copy(Vt[:, ttg * 128:(ttg + 1) * 128],
                                          psv[:])
                for hl in range(HPC):
                    qTh = qT0 if hl == 0 else qT1
                    for b in range(B):
                        qb = b * SB     # q-col base for this batch
                        pO = psO.tile([128, SB], F32, tag="O")
                        acc = ap.tile([128, SB], F32R, tag="acc")
                        for t in range(KT):
                            col0 = 128 * t
                            d = t % 4
                            g0 = t // 4
                            lhsK = kT[:, qb + col0: qb + col0 + 128]
                            bnd = min(1024, SB)
                            tiles = []   # (stile, base, lo, hi)
                            if col0 < bnd:
                                s1 = psS.tile([128, 1024], F32, tag="S")
                                tiles.append((s1, 512 * g0, col0, bnd))
                            if SB > 1024:
                                s2 = psS.tile([128, 1024], F32, tag="S")
                                b2 = max(1024, 512 * g0)
                                tiles.append((s2, b2, max(col0, 1024), SB))
                            for (stile, base, lo, hi) in tiles:
                                for g in range(g0, NG):
                                    glo = max(512 * g, col0)
                                    ghi = 512 * (g + 1)
                                    if ghi <= lo or glo >= hi:
                                        continue
                                    nc.tensor.matmul(
                                        stile[:, glo - base: ghi - base],
                                        lhsK,
                                        qTh[:, qb + glo: qb + ghi],
                                        start=True, stop=True)
                            # exp -> P strip (f32r)
                            P = pstr.tile([128, SB], F32R, tag="P")
                            for (stile, base, lo, hi) in tiles:
                                nc.scalar.activation(
                                    P[:, lo - col0: hi - col0],
                                    stile[:, lo - base: hi - base],
                                    mybir.ActivationFunctionType.Exp,
                                    scale=SCALE)
                            # causal mask on the diagonal block
                            dw = 512 - 128 * d
                            nc.vector.tensor_tensor(
                                P[:, 0:dw], P[:, 0:dw], tri_sb[:, 0:dw],
                                op=mybir.AluOpType.mult)
                            # accumulate exp sums (Pool engine: keeps DVE
                            # off the exp->mask->PV critical path)
                            if t == 0:
                                nc.gpsimd.tensor_copy(acc[:], P[:])
                            else:
                                nc.gpsimd.tensor_tensor(
                                    acc[:, col0:SB], acc[:, col0:SB],
                                    P[:, 0:SB - col0],
                                    op=mybir.AluOpType.add)
                            # P @ V accumulation into O^T
                            lhsV = Vt[:, (b * KT + t) * 128:
                                      (b * KT + t + 1) * 128]
                            for g in range(g0, NG):
                                glo = max(512 * g, col0)
                                ghi = 512 * (g + 1)
                                nc.tensor.matmul(
                                    pO[:, glo:ghi], lhsV,
                                    P[:, glo - col0: ghi - col0],
                                    start=(t == 0),
                                    stop=(t == 4 * g + 3))
                        # epilogue: broadcast sums, fast reciprocal, scale
                        Ofin = ap.tile([128, SB], BF16, tag="Of")
                        for g in range(NG):
                            psr = psS.tile([128, 512], F32, tag="S")
                            nc.tensor.matmul(psr[:], ones_sb[:],
                                             acc[:, 512 * g:512 * (g + 1)],
                                             start=True, stop=True)
                            rb = ap.tile([128, 512], F32, tag="rb")
                            scr = ap.tile([128, 512], F32, tag="scr")
                            nc.vector.reciprocal_approx_accurate(
                                rb[:], psr[:], scr[:])
                            nc.vector.tensor_tensor(
                                Ofin[:, 512 * g:512 * (g + 1)],
                                pO[:, 512 * g:512 * (g + 1)], rb[:],
                                op=mybir.AluOpType.mult)
                        # ship this (b, head) to its a2a dest slots
                        nd = SB // TPC
                        d0 = (b * SB) // TPC
                        for s in range(nd):
                            nc.sync.dma_start(
                                a2a_in[hl][d0 + s, :, :],
                                Ofin[:, s * TPC:(s + 1) * TPC])
                    # per-head collective, overlaps the next head's attention
                    nc.gpsimd.collective_compute(
                        "AllToAll", mybir.AluOpType.bypass,
                        replica_groups=[list(range(N_CORES))],
                        ins=[a2a_in[hl].opt()], outs=[a2a_out[hl].opt()])

        # ---------------- phase 5: O-projection ----------------------
            kks0 = list(range(0, NHD, HPC))      # head-0 hd tiles
            kks1 = list(range(1, NHD, HPC)) if HPC > 1 else []
            with tc.tile_pool(name="oproj", bufs=1) as op, \
                 tc.tile_pool(name="ostg", bufs=2) as ostg, \
                 tc.tile_pool(name="psop", bufs=8, space="PSUM") as pso:
                recv = {}
                for kk in kks0 + kks1:
                    rv = op.tile([128, TPC], BF16, tag=f"rv{kk}")
                    nc.sync.dma_start(rv[:], a2a_out[kk % HPC][kk // HPC, :, :])
                    recv[kk] = rv
                NQO = DIM // DQ
                NTO = TPC // 128
                for wave in range(max(1, NQO // 2)):
                    qs = [q for q in (2 * wave, 2 * wave + 1) if q < NQO]
                    po_map = {}
                    for q in qs:
                        for tt in range(NTO):
                            po = pso.tile([128, DQ], F32, tag="po")
                            po_map[(q, tt)] = po
                            for ki, kk in enumerate(kks0):
                                nc.tensor.matmul(
                                    po[:], recv[kk][:, tt * 128:(tt + 1) * 128],
                                    wo_slice(kk, q * DQ, (q + 1) * DQ),
                                    start=(ki == 0),
                                    stop=(not kks1 and ki == len(kks0) - 1),
                                    skip_group_check=True)
                    for q in qs:
                        for tt in range(NTO):
                            po = po_map[(q, tt)]
                            for ki, kk in enumerate(kks1):
                                nc.tensor.matmul(
                                    po[:], recv[kk][:, tt * 128:(tt + 1) * 128],
                                    wo_slice(kk, q * DQ, (q + 1) * DQ),
                                    start=False, stop=(ki == len(kks1) - 1),
                                    skip_group_check=True)
                            stg = ostg.tile([128, DQ], F32, tag="stg")
                            nc.vector.tensor_copy(stg[:], po[:])
                            nc.sync.dma_start(
                                out_c[tt * 128:(tt + 1) * 128,
                                      q * DQ:(q + 1) * DQ], stg[:])
        wop.release()

    if not nc.is_finalized():
        nc.finalize()
    return nc


_NC_CACHE = {}


def _get_nc():
    if "nc" not in _NC_CACHE:
        _NC_CACHE["nc"] = _build()
    return _NC_CACHE["nc"]


def _prep_inputs(x, cos, sin, wq, wk, wv, wo):
    x = np.asarray(x, np.float32)
    cos = np.asarray(cos, np.float32)
    sin = np.asarray(sin, np.float32)
    wq = np.asarray(wq, np.float32)
    wk = np.asarray(wk, np.float32)
    wv = np.asarray(wv, np.float32)
    wo = np.asarray(wo, np.float32)

    xT = np.ascontiguousarray(x.reshape(T, DIM).T).astype(NPBF16)
    perm = np.r_[np.arange(0, HD, 2), np.arange(1, HD, 2)]
    wq_sh = wq.reshape(DIM, H, HD)[:, :, perm].astype(NPBF16)
    wk_sh = wk.reshape(DIM, HKV, HD)[:, :, perm].astype(NPBF16)
    wv_r = wv.reshape(DIM, HKV, HD).astype(NPBF16)
    wo_b = wo.astype(NPBF16)
    cosT = np.ascontiguousarray(cos.T)          # [64, SB]
    cosd_a = np.vstack([cosT, cosT])            # [128, SB]
    sinT = np.ascontiguousarray(sin.T)
    sind_a = np.vstack([sinT, sinT])
    sgn_a = np.vstack([np.full((64, 1), -1.0, np.float32),
                       np.full((64, 1), 1.0, np.float32)])
    tri_a = (np.arange(512)[None, :] >= np.arange(128)[:, None]
             ).astype(np.float32)
    ones_a = np.ones((128, 128), np.float32)
    ident_a = np.eye(128, dtype=NPBF16)

    in_maps = []
    for c in range(N_CORES):
        h0 = HPC * c
        g = h0 // (H // HKV)
        in_maps.append({
            "xT": xT,
            "wq_c": np.ascontiguousarray(
                wq_sh[:, h0:h0 + HPC].reshape(DIM, QW)),
            "wk_c": np.ascontiguousarray(wk_sh[:, g]),
            "wv_c": np.ascontiguousarray(wv_r[:, g]),
            "wo_f": wo_b,
            "cosd": cosd_a, "sind": sind_a, "sgn": sgn_a, "tri": tri_a,
            "ones": ones_a, "ident": ident_a,
        })
    return in_maps


def _run(inputs, trace=False):
    in_maps = _prep_inputs(**inputs)
    nc = _get_nc()
    res = run_bass_kernel_spmd(
        nc, in_maps, core_ids=list(range(N_CORES)), trace=trace,
        trace_cores=list(range(N_CORES)) if trace else None)
    out = np.concatenate([res.results[c]["out_c"] for c in range(N_CORES)],
                         axis=0)
    return out.reshape(B, SB, DIM), res


def kernel(**inputs):
    out, _ = _run(inputs, trace=os.environ.get("KERNEL_TRACE", "0") == "1")
    return out


# revision 19
# speedup vs baseline: 1.0768x; 1.0768x over previous
"""GQA attention + RoPE + O-proj, tensor-parallel over 8 NeuronCores.

Strategy (head-parallel TP + all-to-all reshard before O-proj):
  - host: transpose x -> xT [DIM, T] in bf16; shuffle per-head wq/wk columns
    to [even hd | odd hd] so RoPE works in the transposed layout; weights in
    bf16 (matmul rate is unchanged vs fp32r, DMA halves).
  - core c: projects q for heads {2c, 2c+1} and k,v for kv-head c//2 over
    all tokens (bf16 weight-stationary matmuls, xT streamed in quarter
    tiles with 2KB lines), applies RoPE inline per token-pair in fp32r,
    transposes V inline, then runs causal attention in S^T [k, q] layout
    with no-max softmax; denominator strip-adds run on the Pool engine
    (DVE was the attention-phase bottleneck), cross-partition sums via
    ones-matmul + fast Newton reciprocal.
  - Two AllToAlls (one per local head, bf16 payload, overlapped with
    attention) reshard attention outputs head-major -> token-sharded; each
    core then computes its 512 output rows against the full wo (bf16,
    fully prefetched during projection/attention).
"""

import os
import numpy as np
import ml_dtypes

import concourse.bass as bass
import concourse.bacc as bacc
import concourse.tile as tile
from concourse import mybir
from concourse.bass_utils import run_bass_kernel_spmd

F32 = mybir.dt.float32
F32R = mybir.dt.float32r
BF16 = mybir.dt.bfloat16
NPBF16 = ml_dtypes.bfloat16

N_CORES = 8

# Full-problem config (hardcoded per spec).
B, SB, DIM = 2, 2048, 2048         # batches, seq per batch, model dim
H, HKV, HD = 16, 4, 128            # q heads, kv heads, head dim
SCALE = 1.0 / float(np.sqrt(HD))

T = B * SB                          # 4096 flat tokens (batch-major)
TPC = T // N_CORES                  # 512 tokens per core (output shard)
HPC = H // N_CORES                  # 2 q heads per core
QW = HPC * HD                       # 256 q cols per core
NKD = DIM // 128                    # 16 contraction tiles for projections
NG = SB // 512                      # 4 q-groups of 512 per batch
KT = SB // 128                      # 16 k-tiles per batch
NTT = T // 128                      # 32 token tiles total
NHD = (H * HD) // 128               # 16 hd row-tiles of wo


def _build():
    nc = bacc.Bacc("TRN2", target_bir_lowering=False, debug=False,
                   num_devices=N_CORES)

    xT = nc.dram_tensor("xT", [DIM, T], BF16, kind="ExternalInput").ap()
    wq_c = nc.dram_tensor("wq_c", [DIM, QW], BF16, kind="ExternalInput").ap()
    wk_c = nc.dram_tensor("wk_c", [DIM, HD], BF16, kind="ExternalInput").ap()
    wv_c = nc.dram_tensor("wv_c", [DIM, HD], BF16, kind="ExternalInput").ap()
    wo_f = nc.dram_tensor("wo_f", [H * HD, DIM], BF16, kind="ExternalInput").ap()
    cosd = nc.dram_tensor("cosd", [128, SB], F32, kind="ExternalInput").ap()
    sind = nc.dram_tensor("sind", [128, SB], F32, kind="ExternalInput").ap()
    sgn = nc.dram_tensor("sgn", [128, 1], F32, kind="ExternalInput").ap()
    tri = nc.dram_tensor("tri", [128, 512], F32, kind="ExternalInput").ap()
    ones = nc.dram_tensor("ones", [128, 128], F32R, kind="ExternalInput").ap()
    ident = nc.dram_tensor("ident", [128, 128], BF16, kind="ExternalInput").ap()
    out_c = nc.dram_tensor("out_c", [TPC, DIM], F32, kind="ExternalOutput").ap()

    a2a_in = []
    a2a_out = []
    for hl in range(HPC):
        a2a_in.append(nc.dram_tensor(f"a2a_in{hl}",
                                     [N_CORES, HD, TPC], BF16).ap())
        a2a_out.append(nc.dram_tensor(f"a2a_out{hl}",
                                      [N_CORES, HD, TPC], BF16).ap())

    SEG = min(512, SB)             # rope segment (never crosses a batch)
    NKQ = max(1, NKD // 4)         # dim-tiles per xt quarter
    NQT = NKD // NKQ               # quarters per token group-pair

    DQH = DIM // 2
    with tile.TileContext(nc) as tc:
        wop = tc.alloc_tile_pool(name="wop", bufs=1)
        wo3 = wo_f.rearrange("(n p) m -> p n m", p=128)      # [128,NHD,DIM]
        wo_h0 = wop.tile([128, NHD * DQH], BF16, tag="wo0")
        wo_h03 = wo_h0.rearrange("p (n m) -> p n m", n=NHD)

        def wo_slice(kk, c0, c1):
            # columns [c0, c1) of wo hd-tile kk, across the two halves
            if c1 <= DQH:
                return wo_h03[:, kk, c0:c1]
            return wo_h13[:, kk, c0 - DQH:c1 - DQH]

        with tc.tile_pool(name="const", bufs=1) as constp, \
             tc.tile_pool(name="qkv", bufs=1) as qkvp:
            sgn_sb = constp.tile([128, 1], F32)
            nc.sync.dma_start(sgn_sb[:], sgn[:, :])

            # persistent roped projections + V in natural layout
            qT0 = qkvp.tile([128, T], F32R, tag="qT0")
            qT1 = qkvp.tile([128, T], F32R, tag="qT1")
            kT = qkvp.tile([128, T], F32R, tag="kT")
            vT = qkvp.tile([128, T], BF16, tag="vT")
            chunks = [qT0, qT1, kT]

            # ------ phase 1: projections + inline RoPE + V transpose ------
            with tc.tile_pool(name="w", bufs=1) as wp, \
                 tc.tile_pool(name="cs", bufs=1) as csp, \
                 tc.tile_pool(name="xt", bufs=4) as xtp, \
                 tc.tile_pool(name="rtmp", bufs=1) as rp, \
                 tc.tile_pool(name="pproj", bufs=1, space="PSUM") as pp:
                wq_sb = wp.tile([128, NKD * QW], BF16)
                wk_sb = wp.tile([128, NKD * HD], BF16)
                wv_sb = wp.tile([128, NKD * HD], BF16)
                # chunked weight loads: kk group 0 lands first so the
                # first matmuls don't wait on whole-tensor DMAs
                wq3d = wq_sb.rearrange("p (n m) -> p n m", n=NKD)
                wk3d = wk_sb.rearrange("p (n m) -> p n m", n=NKD)
                wv3d = wv_sb.rearrange("p (n m) -> p n m", n=NKD)
                wqs = wq_c.rearrange("(n p) m -> p n m", p=128)
                wks = wk_c.rearrange("(n p) m -> p n m", p=128)
                wvs = wv_c.rearrange("(n p) m -> p n m", p=128)
                NWCH = 4
                WCH = NKD // NWCH
                for ch in range(NWCH):
                    k0, k1 = ch * WCH, (ch + 1) * WCH
                    nc.sync.dma_start(wq3d[:, k0:k1, :], wqs[:, k0:k1, :])
                    nc.sync.dma_start(wk3d[:, k0:k1, :], wks[:, k0:k1, :])
                    nc.sync.dma_start(wv3d[:, k0:k1, :], wvs[:, k0:k1, :])

                def w_slice(c, kk):
                    if c < 2:
                        return wq_sb[:, kk * QW + c * 128: kk * QW + (c + 1) * 128]
                    if c == 2:
                        return wk_sb[:, kk * HD:(kk + 1) * HD]
                    return wv_sb[:, kk * HD:(kk + 1) * HD]

                xT3 = xT.rearrange("(n p) m -> p n m", p=128)  # [128,NKD,T]
                npair = T // 1024
                cos_sb = sin_sb = None
                for p in range(npair):
                    # stream this pair's xT as quarter tiles, 1024 tokens
                    # wide (2KB bf16 lines, kk-major use)
                    xts = []            # [quarter] -> [128, NKQ*1024]
                    for q in range(NQT):
                        xt_q = xtp.tile([128, NKQ * 1024], BF16, tag="xt")
                        nc.sync.dma_start(
                            xt_q.rearrange("p (n m) -> p n m", n=NKQ),
                            xT3[:, q * NKQ:(q + 1) * NKQ,
                                p * 1024:(p + 1) * 1024])
                        xts.append(xt_q)
                    if cos_sb is None:
                        cos_sb = csp.tile([128, SB], F32)
                        nc.sync.dma_start(cos_sb[:], cosd[:, :])
                        sin_sb = csp.tile([128, SB], F32)
                        nc.sync.dma_start(sin_sb[:], sind[:, :])
                    pss = []
                    for c in range(4):
                        ps_c = pp.tile([128, 1024], F32, tag=f"pp{c}")
                        pss.append(ps_c)
                    for kk in range(NKD):
                        for c in range(4):
                            lhsT = w_slice(c, kk)
                            xt_q = xts[kk // NKQ]
                            base = (kk % NKQ) * 1024
                            for j in (0, 1):
                                nc.tensor.matmul(
                                    pss[c][:, j * 512:(j + 1) * 512], lhsT,
                                    xt_q[:, base + j * 512: base + (j + 1) * 512],
                                    start=(kk == 0), stop=(kk == NKD - 1))
                    # drain q0/q1/k with RoPE staged below; v via transpose
                    cp0 = 1024 * p
                    for c in range(3):
                        nc.vector.tensor_copy(
                            chunks[c][:, cp0:cp0 + 1024], pss[c][:])
                    nc.vector.tensor_copy(vT[:, cp0:cp0 + 1024], pss[3][:])
                    # prefetch part of wo's first column-half per pair
                    w0 = p * (NHD // npair)
                    w1 = (p + 1) * (NHD // npair)
                    nc.sync.dma_start(wo_h03[:, w0:w1, :],
                                      wo3[:, w0:w1, 0:DQH])
                    # RoPE on the pair's columns, per batch segment
                    for s0 in range(cp0, cp0 + 1024, SEG):
                        pos0 = s0 % SB
                        for X in chunks:
                            tcs = rp.tile([128, SEG], F32, tag="tc")
                            nc.vector.tensor_tensor(
                                tcs[:], X[:, s0:s0 + SEG],
                                cos_sb[:, pos0:pos0 + SEG],
                                op=mybir.AluOpType.mult)
                            tsn = rp.tile([128, SEG], F32, tag="ts")
                            nc.vector.tensor_tensor(
                                tsn[:], X[:, s0:s0 + SEG],
                                sin_sb[:, pos0:pos0 + SEG],
                                op=mybir.AluOpType.mult)
                            tsw = rp.tile([128, SEG], F32, tag="tw")
                            nc.sync.dma_start(tsw[0:64, :], tsn[64:128, :])
                            nc.sync.dma_start(tsw[64:128, :], tsn[0:64, :])
                            # X = tcs + sgn * tsw   (sgn = -1 top / +1 bottom)
                            nc.vector.scalar_tensor_tensor(
                                X[:, s0:s0 + SEG], tsw[:], sgn_sb[:, 0:1],
                                tcs[:], op0=mybir.AluOpType.mult,
                                op1=mybir.AluOpType.add)

            # ---------------- phase 3: attention ----------------------
            DQ = DIM // 4
            with tc.tile_pool(name="att", bufs=2) as ap, \
                 tc.tile_pool(name="attc", bufs=1) as apc, \
                 tc.tile_pool(name="pstr", bufs=2) as pstr, \
                 tc.tile_pool(name="psS", bufs=2, space="PSUM") as psS, \
                 tc.tile_pool(name="psO", bufs=1, space="PSUM") as psO:
                tri_sb = apc.tile([128, 512], F32)
                nc.sync.dma_start(tri_sb[:], tri[:, :])
                ones_sb = apc.tile([128, 128], F32R)
                nc.sync.dma_start(ones_sb[:], ones[:, :])
                ident_sb = apc.tile([128, 128], BF16)
                nc.sync.dma_start(ident_sb[:], ident[:, :])
                # second wo column-half (xt stream pool has freed by now)
                wo_h1 = wop.tile([128, NHD * DQH], BF16, tag="wo1")
                wo_h13 = wo_h1.rearrange("p (n m) -> p n m", n=NHD)
                for ch in range(4):
                    w0 = ch * (NHD // 4)
                    w1 = (ch + 1) * (NHD // 4)
                    nc.sync.dma_start(wo_h13[:, w0:w1, :],
                                      wo3[:, w0:w1, DQH:DIM])
                Vt = qkvp.tile([128, T], F32R, tag="Vt")
                for ttg in range(NTT):
                    psv = psS.tile([128, 128], BF16, tag="S")
                    nc.tensor.transpose(psv[:],
                                        vT[:, ttg * 128:(ttg + 1) * 128],
                                        ident_sb[:])
                    nc.vector.tensor_copy(Vt[:, ttg * 128:(ttg + 1) * 128],
                                          psv[:])
                HW = 1024                 # q-column half width
                for hl in range(HPC):
                    qTh = qT0 if hl == 0 else qT1
                    for b in range(B):
                        qb = b * SB     # q-col base for this batch
                        d0 = (b * SB) // TPC
                        for half in range(SB // HW):
                            h0c = half * HW
                            h1c = h0c + HW
                            pO = psO.tile([128, HW], F32, tag="O")
                            psDen = psO.tile([128, HW], F32, tag="D")
                            tlast = h1c // 128 - 1
                            for t in range(tlast + 1):
                                col0 = 128 * t
                                lo = max(col0, h0c)
                                W = h1c - lo
                                sb0 = 512 * (lo // 512)  # bank-aligned base
                                s0 = lo - sb0
                                lhsK = kT[:, qb + col0: qb + col0 + 128]
                                stile = psS.tile([128, HW], F32, tag="S")
                                for g in range(lo // 512, h1c // 512):
                                    glo = max(512 * g, lo)
                                    ghi = 512 * (g + 1)
                                    nc.tensor.matmul(
                                        stile[:, glo - sb0: ghi - sb0],
                                        lhsK,
                                        qTh[:, qb + glo: qb + ghi],
                                        start=True, stop=True)
                                # exp -> P strip (f32r)
                                P = pstr.tile([128, HW], F32R, tag="P")
                                nc.scalar.activation(
                                    P[:, 0:W], stile[:, s0:s0 + W],
                                    mybir.ActivationFunctionType.Exp,
                                    scale=SCALE)
                                # causal mask: only the first 128 columns
                                # of the diagonal strip can be masked
                                if col0 >= h0c:
                                    nc.vector.tensor_tensor(
                                        P[:, 0:128], P[:, 0:128],
                                        tri_sb[:, 0:128],
                                        op=mybir.AluOpType.mult)
                                # exp-sum accumulation on the PE (PSUM):
                                # every partition of psDen gets the column
                                # sum via the all-ones stationary matrix;
                                # then P @ V accumulation into O^T
                                lhsV = Vt[:, (b * KT + t) * 128:
                                          (b * KT + t + 1) * 128]
                                for g in range(lo // 512, h1c // 512):
                                    glo = max(512 * g, lo)
                                    ghi = 512 * (g + 1)
                                    nc.tensor.matmul(
                                        psDen[:, glo - h0c: ghi - h0c],
                                        ones_sb[:], P[:, glo - lo: ghi - lo],
                                        start=(t == 0),
                                        stop=(t == 4 * g + 3),
                                        skip_group_check=True)
                                    nc.tensor.matmul(
                                        pO[:, glo - h0c: ghi - h0c], lhsV,
                                        P[:, glo - lo: ghi - lo],
                                        start=(t == 0),
                                        stop=(t == 4 * g + 3),
                                        skip_group_check=True)
                            # epilogue: fast reciprocal of exp sums, scale
                            Ofin = ap.tile([128, HW], BF16, tag="Of")
                            for gl in range(HW // 512):
                                rb = ap.tile([128, 512], F32, tag="rb")
                                scr = ap.tile([128, 512], F32, tag="scr")
                                nc.vector.reciprocal_approx_accurate(
                                    rb[:], psDen[:, gl * 512:(gl + 1) * 512],
                                    scr[:])
                                nc.vector.tensor_tensor(
                                    Ofin[:, gl * 512:(gl + 1) * 512],
                                    pO[:, gl * 512:(gl + 1) * 512], rb[:],
                                    op=mybir.AluOpType.mult)
                            # ship this half to its a2a dest slots
                            for s in range(HW // TPC):
                                nc.sync.dma_start(
                                    a2a_in[hl][d0 + (h0c // TPC) + s, :, :],
                                    Ofin[:, s * TPC:(s + 1) * TPC])
                    # per-head collective, overlaps the next head's attention
                    nc.gpsimd.collective_compute(
                        "AllToAll", mybir.AluOpType.bypass,
                        replica_groups=[list(range(N_CORES))],
                        ins=[a2a_in[hl].opt()], outs=[a2a_out[hl].opt()])

        # ---------------- phase 5: O-projection ----------------------
            kks0 = list(range(0, NHD, HPC))      # head-0 hd tiles
            kks1 = list(range(1, NHD, HPC)) if HPC > 1 else []
            with tc.tile_pool(name="oproj", bufs=1) as op, \
                 tc.tile_pool(name="ostg", bufs=2) as ostg, \
                 tc.tile_pool(name="psop", bufs=8, space="PSUM") as pso:
                recv = {}
                for kk in kks0 + kks1:
                    rv = op.tile([128, TPC], BF16, tag=f"rv{kk}")
                    nc.sync.dma_start(rv[:], a2a_out[kk % HPC][kk // HPC, :, :])
                    recv[kk] = rv
                NQO = DIM // DQ
                NTO = TPC // 128
                for wave in range(max(1, NQO // 2)):
                    qs = [q for q in (2 * wave, 2 * wave + 1) if q < NQO]
                    po_map = {}
                    for q in qs:
                        for tt in range(NTO):
                            po = pso.tile([128, DQ], F32, tag="po")
                            po_map[(q, tt)] = po
                            for ki, kk in enumerate(kks0):
                                nc.tensor.matmul(
                                    po[:], recv[kk][:, tt * 128:(tt + 1) * 128],
                                    wo_slice(kk, q * DQ, (q + 1) * DQ),
                                    start=(ki == 0),
                                    stop=(not kks1 and ki == len(kks0) - 1),
                                    skip_group_check=True)
                    for q in qs:
                        for tt in range(NTO):
                            po = po_map[(q, tt)]
                            for ki, kk in enumerate(kks1):
                                nc.tensor.matmul(
                                    po[:], recv[kk][:, tt * 128:(tt + 1) * 128],
                                    wo_slice(kk, q * DQ, (q + 1) * DQ),
                                    start=False, stop=(ki == len(kks1) - 1),
                                    skip_group_check=True)
                            stg = ostg.tile([128, DQ], F32, tag="stg")
                            nc.vector.tensor_copy(stg[:], po[:])
                            nc.sync.dma_start(
                                out_c[tt * 128:(tt + 1) * 128,
                                      q * DQ:(q + 1) * DQ], stg[:])
        wop.release()

    if not nc.is_finalized():
        nc.finalize()
    return nc


_NC_CACHE = {}


def _get_nc():
    if "nc" not in _NC_CACHE:
        _NC_CACHE["nc"] = _build()
    return _NC_CACHE["nc"]


def _prep_inputs(x, cos, sin, wq, wk, wv, wo):
    x = np.asarray(x, np.float32)
    cos = np.asarray(cos, np.float32)
    sin = np.asarray(sin, np.float32)
    wq = np.asarray(wq, np.float32)
    wk = np.asarray(wk, np.float32)
    wv = np.asarray(wv, np.float32)
    wo = np.asarray(wo, np.float32)

    xT = np.ascontiguousarray(x.reshape(T, DIM).T).astype(NPBF16)
    perm = np.r_[np.arange(0, HD, 2), np.arange(1, HD, 2)]
    wq_sh = wq.reshape(DIM, H, HD)[:, :, perm].astype(NPBF16)
    wk_sh = wk.reshape(DIM, HKV, HD)[:, :, perm].astype(NPBF16)
    wv_r = wv.reshape(DIM, HKV, HD).astype(NPBF16)
    wo_b = wo.astype(NPBF16)
    cosT = np.ascontiguousarray(cos.T)          # [64, SB]
    cosd_a = np.vstack([cosT, cosT])            # [128, SB]
    sinT = np.ascontiguousarray(sin.T)
    sind_a = np.vstack([sinT, sinT])
    sgn_a = np.vstack([np.full((64, 1), -1.0, np.float32),
                       np.full((64, 1), 1.0, np.float32)])
    tri_a = (np.arange(512)[None, :] >= np.arange(128)[:, None]
             ).astype(np.float32)
    ones_a = np.ones((128, 128), np.float32)
    ident_a = np.eye(128, dtype=NPBF16)

    in_maps = []
    for c in range(N_CORES):
        h0 = HPC * c
        g = h0 // (H // HKV)
        in_maps.append({
            "xT": xT,
            "wq_c": np.ascontiguousarray(
                wq_sh[:, h0:h0 + HPC].reshape(DIM, QW)),
            "wk_c": np.ascontiguousarray(wk_sh[:, g]),
            "wv_c": np.ascontiguousarray(wv_r[:, g]),
            "wo_f": wo_b,
            "cosd": cosd_a, "sind": sind_a, "sgn": sgn_a, "tri": tri_a,
            "ones": ones_a, "ident": ident_a,
        })
    return in_maps


def _run(inputs, trace=False):
    in_maps = _prep_inputs(**inputs)
    nc = _get_nc()
    res = run_bass_kernel_spmd(
        nc, in_maps, core_ids=list(range(N_CORES)), trace=trace,
        trace_cores=list(range(N_CORES)) if trace else None)
    out = np.concatenate([res.results[c]["out_c"] for c in range(N_CORES)],
                         axis=0)
    return out.reshape(B, SB, DIM), res


def kernel(**inputs):
    out, _ = _run(inputs, trace=os.environ.get("KERNEL_TRACE", "0") == "1")
    return out


# revision 28
# speedup vs baseline: 1.1383x; 1.0571x over previous
"""GQA attention + RoPE + O-proj, tensor-parallel over 8 NeuronCores.

Strategy (head-parallel TP + all-to-all reshard before O-proj):
  - host: transpose x -> xT [DIM, T] in bf16; shuffle per-head wq/wk columns
    to [even hd | odd hd] so RoPE works in the transposed layout; weights in
    bf16 (matmul rate is unchanged vs fp32r, DMA halves).
  - core c: projects q for heads {2c, 2c+1} and k,v for kv-head c//2 over
    all tokens (bf16 weight-stationary matmuls, xT streamed in quarter
    tiles with 2KB lines), applies RoPE inline per token-pair in fp32r,
    transposes V inline, then runs causal attention in S^T [k, q] layout
    with no-max softmax; denominator strip-adds run on the Pool engine
    (DVE was the attention-phase bottleneck), cross-partition sums via
    ones-matmul + fast Newton reciprocal.
  - Two AllToAlls (one per local head, bf16 payload, overlapped with
    attention) reshard attention outputs head-major -> token-sharded; each
    core then computes its 512 output rows against the full wo (bf16,
    fully prefetched during projection/attention).
"""

import os
import numpy as np
import ml_dtypes

import concourse.bass as bass
import concourse.bacc as bacc
import concourse.tile as tile
from concourse import mybir
from concourse.bass_utils import run_bass_kernel_spmd

F32 = mybir.dt.float32
F32R = mybir.dt.float32r
BF16 = mybir.dt.bfloat16
NPBF16 = ml_dtypes.bfloat16

N_CORES = 8

# Full-problem config (hardcoded per spec).
B, SB, DIM = 2, 2048, 2048         # batches, seq per batch, model dim
H, HKV, HD = 16, 4, 128            # q heads, kv heads, head dim
SCALE = 1.0 / float(np.sqrt(HD))

T = B * SB                          # 4096 flat tokens (batch-major)
TPC = T // N_CORES                  # 512 tokens per core (output shard)
HPC = H // N_CORES                  # 2 q heads per core
QW = HPC * HD                       # 256 q cols per core
NKD = DIM // 128                    # 16 contraction tiles for projections
NG = SB // 512                      # 4 q-groups of 512 per batch
KT = SB // 128                      # 16 k-tiles per batch
NTT = T // 128                      # 32 token tiles total
NHD = (H * HD) // 128               # 16 hd row-tiles of wo


def _build():
    nc = bacc.Bacc("TRN2", target_bir_lowering=False, debug=False,
                   num_devices=N_CORES)

    xT = nc.dram_tensor("xT", [DIM, T], BF16, kind="ExternalInput").ap()
    wq_c = nc.dram_tensor("wq_c", [DIM, QW], BF16, kind="ExternalInput").ap()
    wk_c = nc.dram_tensor("wk_c", [DIM, HD], BF16, kind="ExternalInput").ap()
    wv_c = nc.dram_tensor("wv_c", [DIM, HD], BF16, kind="ExternalInput").ap()
    wo_f = nc.dram_tensor("wo_f", [H * HD, DIM], BF16, kind="ExternalInput").ap()
    cosd = nc.dram_tensor("cosd", [128, SB], F32, kind="ExternalInput").ap()
    sind = nc.dram_tensor("sind", [128, SB], F32, kind="ExternalInput").ap()
    sgn = nc.dram_tensor("sgn", [128, 1], F32, kind="ExternalInput").ap()
    tri = nc.dram_tensor("tri", [128, 512], F32, kind="ExternalInput").ap()
    ones = nc.dram_tensor("ones", [128, 128], F32R, kind="ExternalInput").ap()
    ident = nc.dram_tensor("ident", [128, 128], BF16, kind="ExternalInput").ap()
    out_c = nc.dram_tensor("out_c", [TPC, DIM], F32, kind="ExternalOutput").ap()

    a2a_in = []
    a2a_out = []
    for hl in range(HPC):
        a2a_in.append(nc.dram_tensor(f"a2a_in{hl}",
                                     [N_CORES, HD, TPC], BF16).ap())
        a2a_out.append(nc.dram_tensor(f"a2a_out{hl}",
                                      [N_CORES, HD, TPC], BF16).ap())

    SEG = min(512, SB)             # rope segment (never crosses a batch)
    NKQ = max(1, NKD // 4)         # dim-tiles per xt quarter
    NQT = NKD // NKQ               # quarters per token group-pair

    DQH = DIM // 2
    with tile.TileContext(nc) as tc:
        wop = tc.alloc_tile_pool(name="wop", bufs=1)
        wo3 = wo_f.rearrange("(n p) m -> p n m", p=128)      # [128,NHD,DIM]
        wo_h0 = wop.tile([128, NHD * DQH], BF16, tag="wo0")
        wo_h03 = wo_h0.rearrange("p (n m) -> p n m", n=NHD)

        def wo_slice(kk, c0, c1):
            # columns [c0, c1) of wo hd-tile kk, across the two halves
            if c1 <= DQH:
                return wo_h03[:, kk, c0:c1]
            return wo_h13[:, kk, c0 - DQH:c1 - DQH]

        with tc.tile_pool(name="const", bufs=1) as constp, \
             tc.tile_pool(name="qkv", bufs=1) as qkvp:
            sgn_sb = constp.tile([128, 1], F32)
            nc.sync.dma_start(sgn_sb[:], sgn[:, :])

            # persistent roped projections + V in natural layout.
            # One tile per batch so attention on batch 0 doesn't create
            # hazards against batch-1 writes (RoPE tail overlaps).
            qT0 = [qkvp.tile([128, SB], F32R, tag=f"qT0b{b}", name=f"qT0b{b}")
                   for b in range(B)]
            qT1 = [qkvp.tile([128, SB], F32R, tag=f"qT1b{b}", name=f"qT1b{b}")
                   for b in range(B)]
            kT = [qkvp.tile([128, SB], F32R, tag=f"kTb{b}", name=f"kTb{b}")
                  for b in range(B)]
            vT = [qkvp.tile([128, SB], BF16, tag=f"vTb{b}", name=f"vTb{b}")
                  for b in range(B)]
            chunks = [qT0, qT1, kT]

            # ------ phase 1: projections + inline RoPE + V transpose ------
            with tc.tile_pool(name="w", bufs=1) as wp, \
                 tc.tile_pool(name="cs", bufs=1) as csp, \
                 tc.tile_pool(name="xt", bufs=4) as xtp, \
                 tc.tile_pool(name="rtmp", bufs=1) as rp, \
                 tc.tile_pool(name="pproj", bufs=1, space="PSUM") as pp:
                wq_sb = wp.tile([128, NKD * QW], BF16)
                wk_sb = wp.tile([128, NKD * HD], BF16)
                wv_sb = wp.tile([128, NKD * HD], BF16)
                # chunked weight loads: kk group 0 lands first so the
                # first matmuls don't wait on whole-tensor DMAs
                wq3d = wq_sb.rearrange("p (n m) -> p n m", n=NKD)
                wk3d = wk_sb.rearrange("p (n m) -> p n m", n=NKD)
                wv3d = wv_sb.rearrange("p (n m) -> p n m", n=NKD)
                wqs = wq_c.rearrange("(n p) m -> p n m", p=128)
                wks = wk_c.rearrange("(n p) m -> p n m", p=128)
                wvs = wv_c.rearrange("(n p) m -> p n m", p=128)
                NWCH = 4
                WCH = NKD // NWCH
                for ch in range(NWCH):
                    k0, k1 = ch * WCH, (ch + 1) * WCH
                    nc.sync.dma_start(wq3d[:, k0:k1, :], wqs[:, k0:k1, :])
                    nc.sync.dma_start(wk3d[:, k0:k1, :], wks[:, k0:k1, :])
                    nc.sync.dma_start(wv3d[:, k0:k1, :], wvs[:, k0:k1, :])

                def w_slice(c, kk):
                    if c < 2:
                        return wq_sb[:, kk * QW + c * 128: kk * QW + (c + 1) * 128]
                    if c == 2:
                        return wk_sb[:, kk * HD:(kk + 1) * HD]
                    return wv_sb[:, kk * HD:(kk + 1) * HD]

                xT3 = xT.rearrange("(n p) m -> p n m", p=128)  # [128,NKD,T]
                npair = T // 1024
                cos_sb = sin_sb = None
                for p in range(npair):
                    # stream this pair's xT as quarter tiles, 1024 tokens
                    # wide (2KB bf16 lines, kk-major use)
                    xts = []            # [quarter] -> [128, NKQ*1024]
                    for q in range(NQT):
                        xt_q = xtp.tile([128, NKQ * 1024], BF16, tag="xt")
                        nc.sync.dma_start(
                            xt_q.rearrange("p (n m) -> p n m", n=NKQ),
                            xT3[:, q * NKQ:(q + 1) * NKQ,
                                p * 1024:(p + 1) * 1024])
                        xts.append(xt_q)
                    if cos_sb is None:
                        cos_sb = csp.tile([128, SB], F32)
                        nc.sync.dma_start(cos_sb[:], cosd[:, :])
                        sin_sb = csp.tile([128, SB], F32)
                        nc.sync.dma_start(sin_sb[:], sind[:, :])
                    pss = []
                    for c in range(4):
                        ps_c = pp.tile([128, 1024], F32, tag=f"pp{c}")
                        pss.append(ps_c)
                    for kk in range(NKD):
                        for c in range(4):
                            lhsT = w_slice(c, kk)
                            xt_q = xts[kk // NKQ]
                            base = (kk % NKQ) * 1024
                            for j in (0, 1):
                                nc.tensor.matmul(
                                    pss[c][:, j * 512:(j + 1) * 512], lhsT,
                                    xt_q[:, base + j * 512: base + (j + 1) * 512],
                                    start=(kk == 0), stop=(kk == NKD - 1))
                    # drain q0/q1/k with RoPE staged below; v via transpose
                    cb = p // 2            # batch of this pair
                    lcp0 = (p % 2) * 1024  # column base within the batch
                    for c in range(3):
                        nc.vector.tensor_copy(
                            chunks[c][cb][:, lcp0:lcp0 + 1024], pss[c][:])
                    nc.vector.tensor_copy(vT[cb][:, lcp0:lcp0 + 1024],
                                          pss[3][:])
                    # prefetch part of wo's first column-half per pair
                    w0 = p * (NHD // npair)
                    w1 = (p + 1) * (NHD // npair)
                    nc.sync.dma_start(wo_h03[:, w0:w1, :],
                                      wo3[:, w0:w1, 0:DQH])
                    # RoPE on the pair's columns, per batch segment
                    for s0 in range(lcp0, lcp0 + 1024, SEG):
                        for X3 in chunks:
                            X = X3[cb]
                            tcs = rp.tile([128, SEG], F32, tag="tc")
                            nc.vector.tensor_tensor(
                                tcs[:], X[:, s0:s0 + SEG],
                                cos_sb[:, s0:s0 + SEG],
                                op=mybir.AluOpType.mult)
                            tsn = rp.tile([128, SEG], F32, tag="ts")
                            nc.vector.tensor_tensor(
                                tsn[:], X[:, s0:s0 + SEG],
                                sin_sb[:, s0:s0 + SEG],
                                op=mybir.AluOpType.mult)
                            tsw = rp.tile([128, SEG], F32, tag="tw")
                            nc.sync.dma_start(tsw[0:64, :], tsn[64:128, :])
                            nc.sync.dma_start(tsw[64:128, :], tsn[0:64, :])
                            # X = tcs + sgn * tsw   (sgn = -1 top / +1 bottom)
                            nc.vector.scalar_tensor_tensor(
                                X[:, s0:s0 + SEG], tsw[:], sgn_sb[:, 0:1],
                                tcs[:], op0=mybir.AluOpType.mult,
                                op1=mybir.AluOpType.add)

            # ---------------- phase 3: attention ----------------------
            DQ = DIM // 4
            with tc.tile_pool(name="att", bufs=2) as ap, \
                 tc.tile_pool(name="attc", bufs=1) as apc, \
                 tc.tile_pool(name="pstr", bufs=2) as pstr, \
                 tc.tile_pool(name="psS", bufs=2, space="PSUM") as psS, \
                 tc.tile_pool(name="psO", bufs=1, space="PSUM") as psO:
                tri_sb = apc.tile([128, 512], F32)
                nc.sync.dma_start(tri_sb[:], tri[:, :])
                ones_sb = apc.tile([128, 128], F32R)
                nc.sync.dma_start(ones_sb[:], ones[:, :])
                ident_sb = apc.tile([128, 128], BF16)
                nc.sync.dma_start(ident_sb[:], ident[:, :])
                # second wo column-half (xt stream pool has freed by now)
                wo_h1 = wop.tile([128, NHD * DQH], BF16, tag="wo1")
                wo_h13 = wo_h1.rearrange("p (n m) -> p n m", n=NHD)
                for ch in range(4):
                    w0 = ch * (NHD // 4)
                    w1 = (ch + 1) * (NHD // 4)
                    nc.sync.dma_start(wo_h13[:, w0:w1, :],
                                      wo3[:, w0:w1, DQH:DIM])
                Vt = [qkvp.tile([128, SB], F32R, tag=f"Vtb{b}t", name=f"Vtb{b}t")
                      for b in range(B)]
                for b in range(B):
                    for ttg in range(KT):
                        psv = psS.tile([128, 128], BF16, tag="S")
                        nc.tensor.transpose(
                            psv[:], vT[b][:, ttg * 128:(ttg + 1) * 128],
                            ident_sb[:])
                        nc.vector.tensor_copy(
                            Vt[b][:, ttg * 128:(ttg + 1) * 128], psv[:])
                HW = 1024                 # q-column half width
                for hl in range(HPC):
                    for b in range(B):
                        qTh = (qT0 if hl == 0 else qT1)[b]
                        kTb = kT[b]
                        Vtb = Vt[b]
                        d0 = (b * SB) // TPC
                        for half in range(SB // HW):
                            h0c = half * HW
                            h1c = h0c + HW
                            pO = psO.tile([128, HW], F32, tag="O")
                            psDen = psO.tile([128, HW], F32, tag="D")
                            tlast = h1c // 128 - 1
                            for t in range(tlast + 1):
                                col0 = 128 * t
                                lo = max(col0, h0c)
                                W = h1c - lo
                                sb0 = 512 * (lo // 512)  # bank-aligned base
                                s0 = lo - sb0
                                lhsK = kTb[:, col0: col0 + 128]
                                stile = psS.tile([128, HW], F32, tag="S")
                                for g in range(lo // 512, h1c // 512):
                                    glo = max(512 * g, lo)
                                    ghi = 512 * (g + 1)
                                    nc.tensor.matmul(
                                        stile[:, glo - sb0: ghi - sb0],
                                        lhsK,
                                        qTh[:, glo: ghi],
                                        start=True, stop=True)
                                # exp -> P strip (f32r)
                                P = pstr.tile([128, HW], F32R, tag="P")
                                nc.scalar.activation(
                                    P[:, 0:W], stile[:, s0:s0 + W],
                                    mybir.ActivationFunctionType.Exp,
                                    scale=SCALE)
                                # causal mask: only the first 128 columns
                                # of the diagonal strip can be masked
                                if col0 >= h0c:
                                    nc.vector.tensor_tensor(
                                        P[:, 0:128], P[:, 0:128],
                                        tri_sb[:, 0:128],
                                        op=mybir.AluOpType.mult)
                                # exp-sum accumulation on the PE (PSUM):
                                # every partition of psDen gets the column
                                # sum via the all-ones stationary matrix;
                                # then P @ V accumulation into O^T
                                lhsV = Vtb[:, t * 128:(t + 1) * 128]
                                for g in range(lo // 512, h1c // 512):
                                    glo = max(512 * g, lo)
                                    ghi = 512 * (g + 1)
                                    nc.tensor.matmul(
                                        psDen[:, glo - h0c: ghi - h0c],
                                        ones_sb[:], P[:, glo - lo: ghi - lo],
                                        start=(t == 0),
                                        stop=(t == 4 * g + 3),
                                        skip_group_check=True)
                                    nc.tensor.matmul(
                                        pO[:, glo - h0c: ghi - h0c], lhsV,
                                        P[:, glo - lo: ghi - lo],
                                        start=(t == 0),
                                        stop=(t == 4 * g + 3),
                                        skip_group_check=True)
                            # epilogue: fast reciprocal of exp sums, scale
                            Ofin = ap.tile([128, HW], BF16, tag="Of")
                            for gl in range(HW // 512):
                                rb = ap.tile([128, 512], F32, tag="rb")
                                scr = ap.tile([128, 512], F32, tag="scr")
                                nc.vector.reciprocal_approx_accurate(
                                    rb[:], psDen[:, gl * 512:(gl + 1) * 512],
                                    scr[:])
                                nc.vector.tensor_tensor(
                                    Ofin[:, gl * 512:(gl + 1) * 512],
                                    pO[:, gl * 512:(gl + 1) * 512], rb[:],
                                    op=mybir.AluOpType.mult)
                            # ship this half to its a2a dest slots
                            for s in range(HW // TPC):
                                nc.sync.dma_start(
                                    a2a_in[hl][d0 + (h0c // TPC) + s, :, :],
                                    Ofin[:, s * TPC:(s + 1) * TPC])
                    # per-head collective, overlaps the next head's attention
                    nc.gpsimd.collective_compute(
                        "AllToAll", mybir.AluOpType.bypass,
                        replica_groups=[list(range(N_CORES))],
                        ins=[a2a_in[hl].opt()], outs=[a2a_out[hl].opt()])

        # ---------------- phase 5: O-projection ----------------------
            kks0 = list(range(0, NHD, HPC))      # head-0 hd tiles
            kks1 = list(range(1, NHD, HPC)) if HPC > 1 else []
            with tc.tile_pool(name="oproj", bufs=1) as op, \
                 tc.tile_pool(name="ostg", bufs=2) as ostg, \
                 tc.tile_pool(name="psop", bufs=8, space="PSUM") as pso:
                recv = {}
                for kk in kks0 + kks1:
                    rv = op.tile([128, TPC], BF16, tag=f"rv{kk}")
                    nc.sync.dma_start(rv[:], a2a_out[kk % HPC][kk // HPC, :, :])
                    recv[kk] = rv
                NQO = DIM // DQ
                NTO = TPC // 128
                for wave in range(max(1, NQO // 2)):
                    qs = [q for q in (2 * wave, 2 * wave + 1) if q < NQO]
                    po_map = {}
                    for q in qs:
                        for tt in range(NTO):
                            po = pso.tile([128, DQ], F32, tag="po")
                            po_map[(q, tt)] = po
                            for ki, kk in enumerate(kks0):
                                nc.tensor.matmul(
                                    po[:], recv[kk][:, tt * 128:(tt + 1) * 128],
                                    wo_slice(kk, q * DQ, (q + 1) * DQ),
                                    start=(ki == 0),
                                    stop=(not kks1 and ki == len(kks0) - 1),
                                    skip_group_check=True)
                    for q in qs:
                        for tt in range(NTO):
                            po = po_map[(q, tt)]
                            for ki, kk in enumerate(kks1):
                                nc.tensor.matmul(
                                    po[:], recv[kk][:, tt * 128:(tt + 1) * 128],
                                    wo_slice(kk, q * DQ, (q + 1) * DQ),
                                    start=False, stop=(ki == len(kks1) - 1),
                                    skip_group_check=True)
                            stg = ostg.tile([128, DQ], F32, tag="stg")
                            nc.vector.tensor_copy(stg[:], po[:])
                            nc.sync.dma_start(
                                out_c[tt * 128:(tt + 1) * 128,
                                      q * DQ:(q + 1) * DQ], stg[:])
        wop.release()

    if not nc.is_finalized():
        nc.finalize()
    return nc


_NC_CACHE = {}


def _get_nc():
    if "nc" not in _NC_CACHE:
        _NC_CACHE["nc"] = _build()
    return _NC_CACHE["nc"]


def _prep_inputs(x, cos, sin, wq, wk, wv, wo):
    x = np.asarray(x, np.float32)
    cos = np.asarray(cos, np.float32)
    sin = np.asarray(sin, np.float32)
    wq = np.asarray(wq, np.float32)
    wk = np.asarray(wk, np.float32)
    wv = np.asarray(wv, np.float32)
    wo = np.asarray(wo, np.float32)

    xT = np.ascontiguousarray(x.reshape(T, DIM).T).astype(NPBF16)
    perm = np.r_[np.arange(0, HD, 2), np.arange(1, HD, 2)]
    wq_sh = wq.reshape(DIM, H, HD)[:, :, perm].astype(NPBF16)
    wk_sh = wk.reshape(DIM, HKV, HD)[:, :, perm].astype(NPBF16)
    wv_r = wv.reshape(DIM, HKV, HD).astype(NPBF16)
    wo_b = wo.astype(NPBF16)
    cosT = np.ascontiguousarray(cos.T)          # [64, SB]
    cosd_a = np.vstack([cosT, cosT])            # [128, SB]
    sinT = np.ascontiguousarray(sin.T)
    sind_a = np.vstack([sinT, sinT])
    sgn_a = np.vstack([np.full((64, 1), -1.0, np.float32),
                       np.full((64, 1), 1.0, np.float32)])
    tri_a = (np.arange(512)[None, :] >= np.arange(128)[:, None]
             ).astype(np.float32)
    ones_a = np.ones((128, 128), np.float32)
    ident_a = np.eye(128, dtype=NPBF16)

    in_maps = []
    for c in range(N_CORES):
        h0 = HPC * c
        g = h0 // (H // HKV)
        in_maps.append({
            "xT": xT,
            "wq_c": np.ascontiguousarray(
                wq_sh[:, h0:h0 + HPC].reshape(DIM, QW)),
            "wk_c": np.ascontiguousarray(wk_sh[:, g]),
            "wv_c": np.ascontiguousarray(wv_r[:, g]),
            "wo_f": wo_b,
            "cosd": cosd_a, "sind": sind_a, "sgn": sgn_a, "tri": tri_a,
            "ones": ones_a, "ident": ident_a,
        })
    return in_maps


def _run(inputs, trace=False):
    in_maps = _prep_inputs(**inputs)
    nc = _get_nc()
    res = run_bass_kernel_spmd(
        nc, in_maps, core_ids=list(range(N_CORES)), trace=trace,
        trace_cores=list(range(N_CORES)) if trace else None)
    out = np.concatenate([res.results[c]["out_c"] for c in range(N_CORES)],
                         axis=0)
    return out.reshape(B, SB, DIM), res


def kernel(**inputs):
    out, _ = _run(inputs, trace=os.environ.get("KERNEL_TRACE", "0") == "1")
    return out


# revision 30
# speedup vs baseline: 1.1717x; 1.0293x over previous
"""GQA attention + RoPE + O-proj, tensor-parallel over 8 NeuronCores.

Strategy (head-parallel TP + all-to-all reshard before O-proj):
  - host: transpose x -> xT [DIM, T] in bf16; shuffle per-head wq/wk columns
    to [even hd | odd hd] so RoPE works in the transposed layout; weights in
    bf16 (matmul rate is unchanged vs fp32r, DMA halves).
  - core c: projects q for heads {2c, 2c+1} and k,v for kv-head c//2 over
    all tokens (bf16 weight-stationary matmuls, xT streamed in quarter
    tiles with 2KB lines), applies RoPE inline per token-pair in fp32r,
    transposes V inline, then runs causal attention in S^T [k, q] layout
    with no-max softmax; denominator strip-adds run on the Pool engine
    (DVE was the attention-phase bottleneck), cross-partition sums via
    ones-matmul + fast Newton reciprocal.
  - Two AllToAlls (one per local head, bf16 payload, overlapped with
    attention) reshard attention outputs head-major -> token-sharded; each
    core then computes its 512 output rows against the full wo (bf16,
    fully prefetched during projection/attention).
"""

import os
import numpy as np
import ml_dtypes

import concourse.bass as bass
import concourse.bacc as bacc
import concourse.tile as tile
from concourse import mybir
from concourse.bass_utils import run_bass_kernel_spmd

F32 = mybir.dt.float32
F32R = mybir.dt.float32r
BF16 = mybir.dt.bfloat16
NPBF16 = ml_dtypes.bfloat16

N_CORES = 8

# Full-problem config (hardcoded per spec).
B, SB, DIM = 2, 2048, 2048         # batches, seq per batch, model dim
H, HKV, HD = 16, 4, 128            # q heads, kv heads, head dim
SCALE = 1.0 / float(np.sqrt(HD))

T = B * SB                          # 4096 flat tokens (batch-major)
TPC = T // N_CORES                  # 512 tokens per core (output shard)
HPC = H // N_CORES                  # 2 q heads per core
QW = HPC * HD                       # 256 q cols per core
NKD = DIM // 128                    # 16 contraction tiles for projections
NG = SB // 512                      # 4 q-groups of 512 per batch
KT = SB // 128                      # 16 k-tiles per batch
NTT = T // 128                      # 32 token tiles total
NHD = (H * HD) // 128               # 16 hd row-tiles of wo


def _build():
    nc = bacc.Bacc("TRN2", target_bir_lowering=False, debug=False,
                   num_devices=N_CORES)

    xT = nc.dram_tensor("xT", [DIM, T], BF16, kind="ExternalInput").ap()
    wq_c = nc.dram_tensor("wq_c", [DIM, QW], BF16, kind="ExternalInput").ap()
    wk_c = nc.dram_tensor("wk_c", [DIM, HD], BF16, kind="ExternalInput").ap()
    wv_c = nc.dram_tensor("wv_c", [DIM, HD], BF16, kind="ExternalInput").ap()
    wo_f = nc.dram_tensor("wo_f", [H * HD, DIM], BF16, kind="ExternalInput").ap()
    cosd = nc.dram_tensor("cosd", [128, SB], F32, kind="ExternalInput").ap()
    sind = nc.dram_tensor("sind", [128, SB], F32, kind="ExternalInput").ap()
    sgn = nc.dram_tensor("sgn", [128, 1], F32, kind="ExternalInput").ap()
    tri = nc.dram_tensor("tri", [128, 512], BF16, kind="ExternalInput").ap()
    ones = nc.dram_tensor("ones", [128, 128], BF16, kind="ExternalInput").ap()
    out_c = nc.dram_tensor("out_c", [TPC, DIM], F32, kind="ExternalOutput").ap()

    a2a_in = []
    a2a_out = []
    for hl in range(HPC):
        a2a_in.append(nc.dram_tensor(f"a2a_in{hl}",
                                     [N_CORES, HD, TPC], BF16).ap())
        a2a_out.append(nc.dram_tensor(f"a2a_out{hl}",
                                      [N_CORES, HD, TPC], BF16).ap())

    SEG = min(512, SB)             # rope segment (never crosses a batch)
    NKQ = max(1, NKD // 4)         # dim-tiles per xt quarter
    NQT = NKD // NKQ               # quarters per token group-pair

    DQH = DIM // 2
    with tile.TileContext(nc) as tc:
        wop = tc.alloc_tile_pool(name="wop", bufs=1)
        wo3 = wo_f.rearrange("(n p) m -> p n m", p=128)      # [128,NHD,DIM]
        wo_h0 = wop.tile([128, NHD * DQH], BF16, tag="wo0")
        wo_h03 = wo_h0.rearrange("p (n m) -> p n m", n=NHD)

        def wo_slice(kk, c0, c1):
            # columns [c0, c1) of wo hd-tile kk, across the two halves
            if c1 <= DQH:
                return wo_h03[:, kk, c0:c1]
            return wo_h13[:, kk, c0 - DQH:c1 - DQH]

        with tc.tile_pool(name="const", bufs=1) as constp, \
             tc.tile_pool(name="qkv", bufs=1) as qkvp:
            sgn_sb = constp.tile([128, 1], F32)
            nc.sync.dma_start(sgn_sb[:], sgn[:, :])

            # persistent roped projections + V in natural layout.
            # One tile per batch so attention on batch 0 doesn't create
            # hazards against batch-1 writes (RoPE tail overlaps).
            qT0 = [qkvp.tile([128, SB], F32R, tag=f"qT0b{b}", name=f"qT0b{b}")
                   for b in range(B)]
            qT1 = [qkvp.tile([128, SB], F32R, tag=f"qT1b{b}", name=f"qT1b{b}")
                   for b in range(B)]
            kT = [qkvp.tile([128, SB], F32R, tag=f"kTb{b}", name=f"kTb{b}")
                  for b in range(B)]
            vT = [qkvp.tile([128, SB], BF16, tag=f"vTb{b}", name=f"vTb{b}")
                  for b in range(B)]
            Vt = [qkvp.tile([128, SB], BF16, tag=f"Vtb{b}t", name=f"Vtb{b}t")
                  for b in range(B)]
            chunks = [qT0, qT1, kT]

            # ------ phase 1: projections + inline RoPE + V transpose ------
            with tc.tile_pool(name="w", bufs=1) as wp, \
                 tc.tile_pool(name="cs", bufs=1) as csp, \
                 tc.tile_pool(name="xt", bufs=4) as xtp, \
                 tc.tile_pool(name="rtmp", bufs=1) as rp, \
                 tc.tile_pool(name="pproj", bufs=1, space="PSUM") as pp:
                wq_sb = wp.tile([128, NKD * QW], BF16)
                wk_sb = wp.tile([128, NKD * HD], BF16)
                wv_sb = wp.tile([128, NKD * HD], BF16)
                # chunked weight loads: kk group 0 lands first so the
                # first matmuls don't wait on whole-tensor DMAs
                wq3d = wq_sb.rearrange("p (n m) -> p n m", n=NKD)
                wk3d = wk_sb.rearrange("p (n m) -> p n m", n=NKD)
                wv3d = wv_sb.rearrange("p (n m) -> p n m", n=NKD)
                wqs = wq_c.rearrange("(n p) m -> p n m", p=128)
                wks = wk_c.rearrange("(n p) m -> p n m", p=128)
                wvs = wv_c.rearrange("(n p) m -> p n m", p=128)
                NWCH = 4
                WCH = NKD // NWCH
                for ch in range(NWCH):
                    k0, k1 = ch * WCH, (ch + 1) * WCH
                    nc.sync.dma_start(wq3d[:, k0:k1, :], wqs[:, k0:k1, :])
                    nc.sync.dma_start(wk3d[:, k0:k1, :], wks[:, k0:k1, :])
                    nc.sync.dma_start(wv3d[:, k0:k1, :], wvs[:, k0:k1, :])

                def w_slice(c, kk):
                    if c < 2:
                        return wq_sb[:, kk * QW + c * 128: kk * QW + (c + 1) * 128]
                    if c == 2:
                        return wk_sb[:, kk * HD:(kk + 1) * HD]
                    return wv_sb[:, kk * HD:(kk + 1) * HD]

                xT3 = xT.rearrange("(n p) m -> p n m", p=128)  # [128,NKD,T]
                npair = T // 1024
                cos_sb = sin_sb = None
                for p in range(npair):
                    # stream this pair's xT as quarter tiles, 1024 tokens
                    # wide (2KB bf16 lines, kk-major use)
                    xts = []            # [quarter] -> [128, NKQ*1024]
                    for q in range(NQT):
                        xt_q = xtp.tile([128, NKQ * 1024], BF16, tag="xt")
                        nc.sync.dma_start(
                            xt_q.rearrange("p (n m) -> p n m", n=NKQ),
                            xT3[:, q * NKQ:(q + 1) * NKQ,
                                p * 1024:(p + 1) * 1024])
                        xts.append(xt_q)
                    if cos_sb is None:
                        cos_sb = csp.tile([128, SB], F32)
                        nc.sync.dma_start(cos_sb[:], cosd[:, :])
                        sin_sb = csp.tile([128, SB], F32)
                        nc.sync.dma_start(sin_sb[:], sind[:, :])
                    pss = []
                    for c in range(4):
                        ps_c = pp.tile([128, 1024], F32, tag=f"pp{c}")
                        pss.append(ps_c)
                    for kk in range(NKD):
                        for c in range(4):
                            lhsT = w_slice(c, kk)
                            xt_q = xts[kk // NKQ]
                            base = (kk % NKQ) * 1024
                            for j in (0, 1):
                                nc.tensor.matmul(
                                    pss[c][:, j * 512:(j + 1) * 512], lhsT,
                                    xt_q[:, base + j * 512: base + (j + 1) * 512],
                                    start=(kk == 0), stop=(kk == NKD - 1))
                    # drain q0/q1/k with RoPE staged below; v via transpose
                    cb = p // 2            # batch of this pair
                    lcp0 = (p % 2) * 1024  # column base within the batch
                    for c in range(3):
                        nc.vector.tensor_copy(
                            chunks[c][cb][:, lcp0:lcp0 + 1024], pss[c][:])
                    nc.vector.tensor_copy(vT[cb][:, lcp0:lcp0 + 1024],
                                          pss[3][:])
                    # V transpose via the DMA XBAR (no PE/DVE cost)
                    for ttl in range(1024 // 128):
                        c0 = lcp0 + ttl * 128
                        nc.sync.dma_start_transpose(
                            Vt[cb][:, c0:c0 + 128], vT[cb][:, c0:c0 + 128])
                    # prefetch part of wo's first column-half per pair
                    w0 = p * (NHD // npair)
                    w1 = (p + 1) * (NHD // npair)
                    nc.sync.dma_start(wo_h03[:, w0:w1, :],
                                      wo3[:, w0:w1, 0:DQH])
                    # RoPE on the pair's columns, per batch segment
                    for s0 in range(lcp0, lcp0 + 1024, SEG):
                        for X3 in chunks:
                            X = X3[cb]
                            tcs = rp.tile([128, SEG], F32, tag="tc")
                            nc.vector.tensor_tensor(
                                tcs[:], X[:, s0:s0 + SEG],
                                cos_sb[:, s0:s0 + SEG],
                                op=mybir.AluOpType.mult)
                            tsn = rp.tile([128, SEG], F32, tag="ts")
                            nc.vector.tensor_tensor(
                                tsn[:], X[:, s0:s0 + SEG],
                                sin_sb[:, s0:s0 + SEG],
                                op=mybir.AluOpType.mult)
                            tsw = rp.tile([128, SEG], F32, tag="tw")
                            nc.sync.dma_start(tsw[0:64, :], tsn[64:128, :])
                            nc.sync.dma_start(tsw[64:128, :], tsn[0:64, :])
                            # X = tcs + sgn * tsw   (sgn = -1 top / +1 bottom)
                            nc.vector.scalar_tensor_tensor(
                                X[:, s0:s0 + SEG], tsw[:], sgn_sb[:, 0:1],
                                tcs[:], op0=mybir.AluOpType.mult,
                                op1=mybir.AluOpType.add)

            # ---------------- phase 3: attention ----------------------
            DQ = DIM // 4
            with tc.tile_pool(name="att", bufs=2) as ap, \
                 tc.tile_pool(name="attc", bufs=1) as apc, \
                 tc.tile_pool(name="pstr", bufs=2) as pstr, \
                 tc.tile_pool(name="psS", bufs=2, space="PSUM") as psS, \
                 tc.tile_pool(name="psO", bufs=1, space="PSUM") as psO:
                tri_sb = apc.tile([128, 512], BF16)
                nc.sync.dma_start(tri_sb[:], tri[:, :])
                ones_sb = apc.tile([128, 128], BF16)
                nc.sync.dma_start(ones_sb[:], ones[:, :])
                # second wo column-half (xt stream pool has freed by now)
                wo_h1 = wop.tile([128, NHD * DQH], BF16, tag="wo1")
                wo_h13 = wo_h1.rearrange("p (n m) -> p n m", n=NHD)
                for ch in range(4):
                    w0 = ch * (NHD // 4)
                    w1 = (ch + 1) * (NHD // 4)
                    nc.sync.dma_start(wo_h13[:, w0:w1, :],
                                      wo3[:, w0:w1, DQH:DIM])
                HW = 1024                 # q-column half width
                for hl in range(HPC):
                    for b in range(B):
                        qTh = (qT0 if hl == 0 else qT1)[b]
                        kTb = kT[b]
                        Vtb = Vt[b]
                        d0 = (b * SB) // TPC
                        for half in range(SB // HW):
                            h0c = half * HW
                            h1c = h0c + HW
                            pO = psO.tile([128, HW], F32, tag="O")
                            psDen = psO.tile([128, HW], F32, tag="D")
                            tlast = h1c // 128 - 1
                            for t in range(tlast + 1):
                                col0 = 128 * t
                                lo = max(col0, h0c)
                                W = h1c - lo
                                sb0 = 512 * (lo // 512)  # bank-aligned base
                                s0 = lo - sb0
                                lhsK = kTb[:, col0: col0 + 128]
                                stile = psS.tile([128, HW], F32, tag="S")
                                for g in range(lo // 512, h1c // 512):
                                    glo = max(512 * g, lo)
                                    ghi = 512 * (g + 1)
                                    nc.tensor.matmul(
                                        stile[:, glo - sb0: ghi - sb0],
                                        lhsK,
                                        qTh[:, glo: ghi],
                                        start=True, stop=True)
                                # exp -> P strip (f32r)
                                P = pstr.tile([128, HW], BF16, tag="P")
                                nc.scalar.activation(
                                    P[:, 0:W], stile[:, s0:s0 + W],
                                    mybir.ActivationFunctionType.Exp,
                                    scale=SCALE)
                                # causal mask: only the first 128 columns
                                # of the diagonal strip can be masked
                                if col0 >= h0c:
                                    nc.vector.tensor_tensor(
                                        P[:, 0:128], P[:, 0:128],
                                        tri_sb[:, 0:128],
                                        op=mybir.AluOpType.mult)
                                # exp-sum accumulation on the PE (PSUM):
                                # every partition of psDen gets the column
                                # sum via the all-ones stationary matrix;
                                # then P @ V accumulation into O^T
                                lhsV = Vtb[:, t * 128:(t + 1) * 128]
                                for g in range(lo // 512, h1c // 512):
                                    glo = max(512 * g, lo)
                                    ghi = 512 * (g + 1)
                                    nc.tensor.matmul(
                                        psDen[:, glo - h0c: ghi - h0c],
                                        ones_sb[:], P[:, glo - lo: ghi - lo],
                                        start=(t == 0),
                                        stop=(t == 4 * g + 3),
                                        skip_group_check=True)
                                    nc.tensor.matmul(
                                        pO[:, glo - h0c: ghi - h0c], lhsV,
                                        P[:, glo - lo: ghi - lo],
                                        start=(t == 0),
                                        stop=(t == 4 * g + 3),
                                        skip_group_check=True)
                                # fused epilogue: when a 512-col group's
                                # den/PV accumulation stops (t == 4g+3),
                                # normalize and ship it while later tiles
                                # keep the PE busy
                                if t % 4 == 3 and t // 4 >= h0c // 512:
                                    gd = t // 4
                                    gl = gd - h0c // 512
                                    rb = ap.tile([128, 512], F32, tag="rb")
                                    scr = ap.tile([128, 512], F32, tag="scr")
                                    nc.vector.reciprocal_approx_accurate(
                                        rb[:],
                                        psDen[:, gl * 512:(gl + 1) * 512],
                                        scr[:])
                                    Ofin = ap.tile([128, 512], BF16, tag="Of")
                                    nc.vector.tensor_tensor(
                                        Ofin[:],
                                        pO[:, gl * 512:(gl + 1) * 512], rb[:],
                                        op=mybir.AluOpType.mult)
                                    nc.sync.dma_start(
                                        a2a_in[hl][d0 + gd, :, :], Ofin[:])
                    # per-head collective, overlaps the next head's attention
                    nc.gpsimd.collective_compute(
                        "AllToAll", mybir.AluOpType.bypass,
                        replica_groups=[list(range(N_CORES))],
                        ins=[a2a_in[hl].opt()], outs=[a2a_out[hl].opt()])

        # ---------------- phase 5: O-projection ----------------------
            kks0 = list(range(0, NHD, HPC))      # head-0 hd tiles
            kks1 = list(range(1, NHD, HPC)) if HPC > 1 else []
            with tc.tile_pool(name="oproj", bufs=1) as op, \
                 tc.tile_pool(name="ostg", bufs=2) as ostg, \
                 tc.tile_pool(name="psop", bufs=8, space="PSUM") as pso:
                recv = {}
                for kk in kks0 + kks1:
                    rv = op.tile([128, TPC], BF16, tag=f"rv{kk}")
                    nc.sync.dma_start(rv[:], a2a_out[kk % HPC][kk // HPC, :, :])
                    recv[kk] = rv
                NQO = DIM // DQ
                NTO = TPC // 128
                for wave in range(max(1, NQO // 2)):
                    qs = [q for q in (2 * wave, 2 * wave + 1) if q < NQO]
                    po_map = {}
                    for q in qs:
                        for tt in range(NTO):
                            po = pso.tile([128, DQ], F32, tag="po")
                            po_map[(q, tt)] = po
                            for ki, kk in enumerate(kks0):
                                nc.tensor.matmul(
                                    po[:], recv[kk][:, tt * 128:(tt + 1) * 128],
                                    wo_slice(kk, q * DQ, (q + 1) * DQ),
                                    start=(ki == 0),
                                    stop=(not kks1 and ki == len(kks0) - 1),
                                    skip_group_check=True)
                    for q in qs:
                        for tt in range(NTO):
                            po = po_map[(q, tt)]
                            for ki, kk in enumerate(kks1):
                                nc.tensor.matmul(
                                    po[:], recv[kk][:, tt * 128:(tt + 1) * 128],
                                    wo_slice(kk, q * DQ, (q + 1) * DQ),
                                    start=False, stop=(ki == len(kks1) - 1),
                                    skip_group_check=True)
                            stg = ostg.tile([128, DQ], F32, tag="stg")
                            nc.vector.tensor_copy(stg[:], po[:])
                            nc.sync.dma_start(
                                out_c[tt * 128:(tt + 1) * 128,
                                      q * DQ:(q + 1) * DQ], stg[:])
        wop.release()

    if not nc.is_finalized():
        nc.finalize()
    return nc


_NC_CACHE = {}


def _get_nc():
    if "nc" not in _NC_CACHE:
        _NC_CACHE["nc"] = _build()
    return _NC_CACHE["nc"]


def _prep_inputs(x, cos, sin, wq, wk, wv, wo):
    x = np.asarray(x, np.float32)
    cos = np.asarray(cos, np.float32)
    sin = np.asarray(sin, np.float32)
    wq = np.asarray(wq, np.float32)
    wk = np.asarray(wk, np.float32)
    wv = np.asarray(wv, np.float32)
    wo = np.asarray(wo, np.float32)

    xT = np.ascontiguousarray(x.reshape(T, DIM).T).astype(NPBF16)
    perm = np.r_[np.arange(0, HD, 2), np.arange(1, HD, 2)]
    wq_sh = wq.reshape(DIM, H, HD)[:, :, perm].astype(NPBF16)
    wk_sh = wk.reshape(DIM, HKV, HD)[:, :, perm].astype(NPBF16)
    wv_r = wv.reshape(DIM, HKV, HD).astype(NPBF16)
    wo_b = wo.astype(NPBF16)
    cosT = np.ascontiguousarray(cos.T)          # [64, SB]
    cosd_a = np.vstack([cosT, cosT])            # [128, SB]
    sinT = np.ascontiguousarray(sin.T)
    sind_a = np.vstack([sinT, sinT])
    sgn_a = np.vstack([np.full((64, 1), -1.0, np.float32),
                       np.full((64, 1), 1.0, np.float32)])
    tri_a = (np.arange(512)[None, :] >= np.arange(128)[:, None]
             ).astype(NPBF16)
    ones_a = np.ones((128, 128), NPBF16)

    in_maps = []
    for c in range(N_CORES):
        h0 = HPC * c
        g = h0 // (H // HKV)
        in_maps.append({
            "xT": xT,
            "wq_c": np.ascontiguousarray(
                wq_sh[:, h0:h0 + HPC].reshape(DIM, QW)),
            "wk_c": np.ascontiguousarray(wk_sh[:, g]),
            "wv_c": np.ascontiguousarray(wv_r[:, g]),
            "wo_f": wo_b,
            "cosd": cosd_a, "sind": sind_a, "sgn": sgn_a, "tri": tri_a,
            "ones": ones_a,
        })
    return in_maps


def _run(inputs, trace=False):
    in_maps = _prep_inputs(**inputs)
    nc = _get_nc()
    res = run_bass_kernel_spmd(
        nc, in_maps, core_ids=list(range(N_CORES)), trace=trace,
        trace_cores=list(range(N_CORES)) if trace else None)
    out = np.concatenate([res.results[c]["out_c"] for c in range(N_CORES)],
                         axis=0)
    return out.reshape(B, SB, DIM), res


def kernel(**inputs):
    out, _ = _run(inputs, trace=os.environ.get("KERNEL_TRACE", "0") == "1")
    return out
